# revision 1
# baseline (speedup 1.0000x reference)
"""Single-head attention Trainium2 kernel (batch=8 data-parallel over 8 cores).

Reference computation (per batch element b):
    Q = x @ Wq; K = x @ Wk; V = x @ Wv          (x: [S, D], W*: [D, O])
    out = softmax(Q @ K.T * SCALE) @ V          (SCALE = 1/8, hardcoded sqrt(64))

Kernel strategy (per core, one batch element):
  Phase A: x -> xT via PE transposes (d on partitions).
  Phase B: QT = Wq.T-style matmuls (QT[o,s] streamed to DRAM scratch),
           KT[o,s] kept resident in SBUF.
  Phase C (per q-block): scoresT[ks,q] = KT.T-chunks @ QT-block,
           expT = exp(SCALE*scoresT)  (ACT, fused scale),
           row-sums via ones-matmul -> reciprocal,
           A^T[d,q] = x-chunks.T @ expT   (reassociation: attn @ x),
           out[q,o] = (A^T).T @ Wv, normalized by reciprocal on eviction.
  All matmuls run in float32r (full PE rate at N>=256, ~1e-4 accuracy).
"""

import sys

sys.path.insert(0, "/opt/trn_rl_repo")

from contextlib import ExitStack

import numpy as np

import concourse.bass as bass
import concourse.mybir as mybir
from concourse import bacc
from concourse.tile import TileContext
from concourse.masks import make_identity

F32 = mybir.dt.float32
F32R = mybir.dt.float32r
EXP = mybir.ActivationFunctionType.Exp
SCALE = 1.0 / 8.0  # 1/sqrt(64), hardcoded in the reference module


def build_attn(S=2048, D=1024, O=1024, QB=256, compute_dtype=F32R, reps=1, phases='abcuvo'):
    """Build the Bass module for one core: x[S,D], w[3,D,O] -> out[S,O].

    reps>1 repeats the whole computation serially (timing slope method).
    """
    CD = compute_dtype
    SB = 512  # s-block width for phase B outputs
    NSB = S // SB
    DC = D // 128
    OC = O // 128
    KC = S // 128
    NQB = S // QB
    QC = QB // 128
    OH = (O + 511) // 512
    OHW = min(O, 512)

    nc = bacc.Bacc("TRN2", target_bir_lowering=False, debug=False)
    x_in = nc.dram_tensor("x", [S, D], F32, kind="ExternalInput")
    w_in = nc.dram_tensor("w", [3, D, O], F32, kind="ExternalInput")
    out_d = nc.dram_tensor("out", [S, O], F32, kind="ExternalOutput")

    def cast(ap):
        return ap.bitcast(CD) if CD != F32 else ap

    with TileContext(nc) as tc:
      for _rep in range(reps):
        top = ExitStack()
        dram = top.enter_context(tc.tile_pool(name="dram", bufs=1, space="DRAM"))
        qt_t = dram.tile([O, S], F32)

        kt_pool = top.enter_context(tc.tile_pool(name="ktp", bufs=OC * NSB))
        const_pool = top.enter_context(tc.tile_pool(name="constp", bufs=1))

        # identity for PE transposes (in compute dtype)
        ident_f = const_pool.tile([128, 128], F32, tag="identf")
        make_identity(nc, ident_f)
        if CD != F32:
            ident = const_pool.tile([128, 128], CD, tag="identr")
            nc.vector.tensor_copy(out=ident, in_=ident_f)
        else:
            ident = ident_f
        # ones column for row-sum matmuls
        ones_f = const_pool.tile([128, 1], F32, tag="onesf")
        nc.gpsimd.memset(ones_f, 1.0)
        if CD != F32:
            ones = const_pool.tile([128, 1], CD, tag="onesr")
            nc.vector.tensor_copy(out=ones, in_=ones_f)
        else:
            ones = ones_f

        kt = [[None] * NSB for _ in range(OC)]

        with ExitStack() as ph_ab:
            xn_pool = ph_ab.enter_context(tc.tile_pool(name="xnp", bufs=6))
            xt_pool = ph_ab.enter_context(tc.tile_pool(name="xtp", bufs=DC * NSB))
            w_pool = ph_ab.enter_context(tc.tile_pool(name="wp", bufs=4 * DC))
            qs_pool = ph_ab.enter_context(tc.tile_pool(name="qsp", bufs=4))
            psA = ph_ab.enter_context(tc.tile_pool(name="psA", bufs=4, space="PSUM"))
            psB = ph_ab.enter_context(tc.tile_pool(name="psB", bufs=4, space="PSUM"))

            # ---- Phases A+B interleaved in two halves: transposes for
            # half h+1 overlap the projection matmuls of half h (PE stays
            # busy while x streams in). Weights are streamed once per half.
            xt = [[None] * NSB for _ in range(DC)]
            xn_sb = [None] * (SB // 128)
            # two halves: second half's transposes overlap first half's
            # projection matmuls (weights streamed once per half)
            _mid = (NSB + 1) // 2
            _halves = [h for h in (list(range(0, _mid)), list(range(_mid, NSB))) if h]
            for half in range(len(_halves) if 'b' in phases else 0):
                sbs = _halves[half]
                # -- transposes for this half's s-blocks --
                for sb in sbs:
                    for ss in range(SB // 128):
                        kc = sb * (SB // 128) + ss
                        xn_t = xn_pool.tile([128, D], CD, tag="xn", bufs=6)
                        dma_eng = nc.sync if kc % 2 == 0 else nc.gpsimd
                        dma_eng.dma_start(
                            out=xn_t, in_=cast(x_in[kc * 128 : (kc + 1) * 128, :])
                        )
                        xn_sb[ss] = xn_t
                    for dc in range(DC):
                        ps = psA.tile([128, SB], CD, tag="pst", bufs=4)
                        for s2 in range(SB // 128):
                            nc.tensor.transpose(
                                ps[:, s2 * 128 : (s2 + 1) * 128],
                                xn_sb[s2][:, dc * 128 : (dc + 1) * 128],
                                ident,
                            )
                        xt[dc][sb] = xt_pool.tile(
                            [128, SB], CD, tag="xt", bufs=DC * NSB, name=f"xt_{dc}_{sb}"
                        )
                        nc.vector.tensor_copy(out=xt[dc][sb], in_=ps)
                # -- projections for this half's s-blocks --
                for oc in range(OC):
                    wq_t = w_pool.tile(
                        [128, DC * 128], CD, tag="wq", bufs=3, name=f"wq_{half}_{oc}"
                    )
                    nc.sync.dma_start(
                        out=wq_t.rearrange("p (c o) -> p c o", c=DC),
                        in_=cast(w_in[0, :, oc * 128 : (oc + 1) * 128]).rearrange(
                            "(c p) o -> p c o", p=128
                        ),
                    )
                    wk_t = w_pool.tile(
                        [128, DC * 128], CD, tag="wk", bufs=3, name=f"wk_{half}_{oc}"
                    )
                    nc.sync.dma_start(
                        out=wk_t.rearrange("p (c o) -> p c o", c=DC),
                        in_=cast(w_in[1, :, oc * 128 : (oc + 1) * 128]).rearrange(
                            "(c p) o -> p c o", p=128
                        ),
                    )
                    wq_col = [wq_t[:, dc * 128 : (dc + 1) * 128] for dc in range(DC)]
                    wk_col = [wk_t[:, dc * 128 : (dc + 1) * 128] for dc in range(DC)]
                    qs = qs_pool.tile(
                        [128, SB * len(sbs)], F32, tag="qts", bufs=2, name=f"qts_{half}_{oc}"
                    )
                    for j, sb in enumerate(sbs):
                        ps_q = psB.tile([128, SB], F32, tag="psb", bufs=4)
                        for dc in range(DC):
                            nc.tensor.matmul(
                                ps_q, wq_col[dc], xt[dc][sb],
                                start=(dc == 0), stop=(dc == DC - 1),
                            )
                        nc.vector.tensor_copy(
                            out=qs[:, j * SB : (j + 1) * SB], in_=ps_q
                        )
                        ps_k = psB.tile([128, SB], F32, tag="psb", bufs=4)
                        for dc in range(DC):
                            nc.tensor.matmul(
                                ps_k, wk_col[dc], xt[dc][sb],
                                start=(dc == 0), stop=(dc == DC - 1),
                            )
                        kt_tile = kt_pool.tile(
                            [128, SB], CD, tag="kt", bufs=OC * NSB, name=f"kt_{oc}_{sb}"
                        )
                        nc.scalar.copy(out=kt_tile, in_=ps_k)
                        kt[oc][sb] = kt_tile
                    nc.sync.dma_start(
                        out=qt_t[
                            oc * 128 : (oc + 1) * 128,
                            sbs[0] * SB : (sbs[-1] + 1) * SB,
                        ],
                        in_=qs,
                    )

        # ---- Phase C: attention per q-block ----
        with ExitStack() as ph_c:
          if 'c' in phases:
                xn2_pool = ph_c.enter_context(tc.tile_pool(name="xn2p", bufs=KC))
                wv_pool = ph_c.enter_context(tc.tile_pool(name="wvp", bufs=DC))
                qtin_pool = ph_c.enter_context(tc.tile_pool(name="qtinp", bufs=OC + 2))
                exp_pool = ph_c.enter_context(tc.tile_pool(name="expp", bufs=KC + 1))
                at_pool = ph_c.enter_context(tc.tile_pool(name="atp", bufs=DC))
                outs_pool = ph_c.enter_context(tc.tile_pool(name="outsp", bufs=2))
                small_pool = ph_c.enter_context(tc.tile_pool(name="smallp", bufs=4 * QC))
                pcs = ph_c.enter_context(tc.tile_pool(name="pcs", bufs=3, space="PSUM"))
                pcsum = ph_c.enter_context(tc.tile_pool(name="pcsum", bufs=1, space="PSUM"))
                pca = ph_c.enter_context(tc.tile_pool(name="pca", bufs=2, space="PSUM"))
                pco = ph_c.enter_context(tc.tile_pool(name="pco", bufs=2, space="PSUM"))

                xn2 = []
                for kc in range(KC):
                    t = xn2_pool.tile([128, D], CD, tag="xn2", bufs=KC, name=f"xn2_{kc}")
                    nc.gpsimd.dma_start(out=t, in_=cast(x_in[kc * 128 : (kc + 1) * 128, :]))
                    xn2.append(t)
                wv = []
                for dc in range(DC):
                    t = wv_pool.tile([128, O], CD, tag="wv", bufs=DC, name=f"wv_{dc}")
                    nc.gpsimd.dma_start(out=t, in_=cast(w_in[2, dc * 128 : (dc + 1) * 128, :]))
                    wv.append(t)

                for qb in range(NQB):
                    q0 = qb * QB
                    qt_blk = qtin_pool.tile(
                        [128, OC * QB], CD, tag="qtin", bufs=2, name=f"qtin_{qb}"
                    )
                    if qb == 0:
                        for oc in range(OC):
                            nc.sync.dma_start(
                                out=qt_blk[:, oc * QB : (oc + 1) * QB],
                                in_=cast(
                                    qt_t[oc * 128 : (oc + 1) * 128, q0 : q0 + QB]
                                ),
                            )
                    else:
                        nc.sync.dma_start(
                            out=qt_blk.rearrange("p (c q) -> p c q", c=OC),
                            in_=cast(qt_t[:, q0 : q0 + QB]).rearrange(
                                "(c p) q -> p c q", p=128
                            ),
                        )
                    qts = [qt_blk[:, oc * QB : (oc + 1) * QB] for oc in range(OC)]
                    # scoresT[ks, q]: two kc-chunks share one PSUM bank as
                    # independent accumulation groups; one wide exp per pair.
                    exp_pairs = []
                    for kp in range(KC // 2):
                        ps_s = pcs.tile([128, 2 * QB], F32, tag="pcs", bufs=3)
                        for half in range(2):
                            kc = 2 * kp + half
                            sb, ss = kc // (SB // 128), kc % (SB // 128)
                            dst = ps_s[:, half * QB : (half + 1) * QB]
                            for oc in range(OC):
                                nc.tensor.matmul(
                                    dst,
                                    kt[oc][sb][:, ss * 128 : (ss + 1) * 128],
                                    qts[oc],
                                    start=(oc == 0), stop=(oc == OC - 1),
                                )
                        e = exp_pool.tile([128, 2 * QB], CD, tag="expT", bufs=KC // 2 + 1)
                        nc.scalar.activation(out=e, in_=ps_s, func=EXP, scale=SCALE)
                        exp_pairs.append(e)
                    expT = [
                        exp_pairs[kc // 2][:, (kc % 2) * QB : (kc % 2 + 1) * QB]
                        for kc in range(KC)
                    ]
                    # A^T[d, q] = sum_ks x[ks, d] * expT[ks, q]
                    # two dc-chunks share one PSUM bank as independent
                    # accumulation groups; one wide DVE copy evicts both.
                    aT_pairs = []
                    for dp in range((DC if 'v' in phases else 0) // 2):
                        ps_a = pca.tile([128, 2 * QB], F32, tag="pca", bufs=2)
                        for half in range(2):
                            dc = 2 * dp + half
                            dst = ps_a[:, half * QB : (half + 1) * QB]
                            for kc in range(KC):
                                nc.tensor.matmul(
                                    dst,
                                    xn2[kc][:, dc * 128 : (dc + 1) * 128],
                                    expT[kc],
                                    start=(kc == 0), stop=(kc == KC - 1),
                                )
                        a_t = at_pool.tile([128, 2 * QB], CD, tag="aT", bufs=DC // 2)
                        nc.vector.tensor_copy(out=a_t, in_=ps_a)
                        aT_pairs.append(a_t)
                    aT = [
                        aT_pairs[dc // 2][:, (dc % 2) * QB : (dc % 2 + 1) * QB]
                        for dc in range(DC if 'v' in phases else 0)
                    ]
                    # row sums (over ks = partitions) via ones-matmul, then 1/x
                    recips = []
                    for qc in range(QC if 'u' in phases else 0):
                        ps_sum = pcsum.tile([128, 1], F32, tag="pcsum", bufs=1)
                        for kc in range(KC):
                            # N=1 matmuls are invalid ISA in float32r; run them as
                            # plain fp32 on the same bits (cheap at N=1).
                            nc.tensor.matmul(
                                ps_sum,
                                expT[kc][:, qc * 128 : (qc + 1) * 128].bitcast(F32),
                                ones_f,
                                start=(kc == 0), stop=(kc == KC - 1),
                            )
                        rc = small_pool.tile([128, 1], F32, tag="recip", bufs=4 * QC)
                        nc.vector.reciprocal(out=rc, in_=ps_sum)
                        recips.append(rc)
                    # out[q, o] = A @ Wv, normalized
                    for qc in range(QC if 'o' in phases else 0):
                        for oh in range(OH):
                            ps_o = pco.tile([128, OHW], F32, tag="pco", bufs=2)
                            for dc in range(DC):
                                nc.tensor.matmul(
                                    ps_o,
                                    aT[dc][:, qc * 128 : (qc + 1) * 128],
                                    wv[dc][:, oh * OHW : (oh + 1) * OHW],
                                    start=(dc == 0), stop=(dc == DC - 1),
                                )
                            os_ = outs_pool.tile([128, OHW], F32, tag="outs", bufs=2)
                            nc.vector.tensor_scalar_mul(out=os_, in0=ps_o, scalar1=recips[qc])
                            nc.sync.dma_start(
                                out=out_d[
                                    q0 + qc * 128 : q0 + (qc + 1) * 128,
                                    oh * OHW : (oh + 1) * OHW,
                                ],
                                in_=os_,
                            )

        top.close()

    nc.compile()
    return nc


_NC_CACHE = {}


def _get_nc():
    key = "full"
    if key not in _NC_CACHE:
        _NC_CACHE[key] = build_attn()
    return _NC_CACHE[key]


def kernel(**inputs):
    """Full-input entry point: x [8, 2048, 1024], kernel [3, 1024, 1024]."""
    from concourse.bass_utils import run_bass_kernel_spmd

    x = np.ascontiguousarray(inputs["x"], dtype=np.float32)
    w = np.ascontiguousarray(inputs["kernel"], dtype=np.float32)
    B = x.shape[0]
    nc = _get_nc()
    in_maps = [{"x": x[b], "w": w} for b in range(B)]
    res = run_bass_kernel_spmd(nc, in_maps, core_ids=list(range(B)))
    return np.stack([res.results[b]["out"] for b in range(B)], axis=0)

